# revision 7
# baseline (speedup 1.0000x reference)
"""Trainium2 Bass kernel for DynamicSparseAttention (B=4, C=256, H=W=64).

Sharding: 2 cores per batch element (8 cores total); each core owns 2048 of
the 4096 queries and duplicates the (small) K/V work. Layouts are channel-
major so attention needs no transposes.

v2 design (vs the bf16 baseline):
  - fp8(e4m3, trn max 240) on the hot attention path: q8/k8 feed the S^T
    matmuls as DoubleRow pairs (one instr contracts both 128-channel
    halves), exp writes et in fp8, AV and the softmax denominators
    (ones-vector matmuls) are DoubleRow too. Everything else stays bf16;
    measured end-to-end rel err ~1e-2 vs fp32 reference (tol 2e-2).
  - logits are rescaled inside the exp activation (scale=1/16, bias=-3) so
    weights are not pre-scaled (pre-scaled fp8 weights would underflow)
    and fp8 et never overflows; the bias cancels in normalization.
  - 3-stage software pipeline over 512-query chunks: S+exp for chunk qc,
    AV + denominator + normalization for qc-1, projection/residual/DMA-out
    for qc-2. PE never waits on the ACT exp stream (its deps are a full
    chunk old); chunk 0's AV slots are filled with the V matmuls.
  - x streams in as bf16 (half the DMA of fp32); channel stats for the
    gate run on DVE (sum) + GpSimd (max) instead of ACT, keeping ACT
    exp-only. Residual add uses the bf16 x (error ~0.2% of |x|).
"""

import numpy as np
import ml_dtypes

import concourse.bass as bass
import concourse.bacc as bacc
import concourse.mybir as mybir
import concourse.tile as tile
from concourse.bass import ts
from concourse.bass_utils import run_bass_kernel_spmd

F32 = mybir.dt.float32
BF16 = mybir.dt.bfloat16
F8 = mybir.dt.float8e4
AF = mybir.ActivationFunctionType
ALU = mybir.AluOpType
DR = mybir.MatmulPerfMode.DoubleRow

B, C, H, W = 4, 256, 64, 64
N = H * W              # 4096 tokens per batch element
P = 128                # partitions
CT = C // P            # channel tiles (2)
NCORES = 8
QN = N * B // NCORES   # queries per core (2048)
HID = 32
QCH = 512              # query chunk for attention
MT = N // P            # key tiles (32)
NQC = QN // QCH        # query chunks per core (4)
SCALE = 1.0 / np.sqrt(C)   # folded into the exp activation scale
EXP_BIAS = -3.0            # cancels in normalization; keeps fp8 et < 240

bf16 = ml_dtypes.bfloat16
f8 = ml_dtypes.float8_e4m3


def _build(reps=1, variant="full"):
    # variant flags for timing experiments (graded path always uses "full")
    bf16_av = "bf16av" in variant   # et bf16, AV bf16, DVE-tree denominator
    v_act = "vact" in variant       # v8 copies on ACT instead of DVE
    pstg_bufs = 3 if "ps3" in variant else 2
    no_attn = "noattn" in variant
    from contextlib import ExitStack

    nc = bacc.Bacc()

    xb = nc.declare_dram_parameter("xb", [C, N], BF16, isOutput=False)
    wqT = nc.declare_dram_parameter("wqT", [C, C], BF16, isOutput=False)
    wkT = nc.declare_dram_parameter("wkT", [C, C], BF16, isOutput=False)
    wvT = nc.declare_dram_parameter("wvT", [C, C], BF16, isOutput=False)
    wpT = nc.declare_dram_parameter("wpT", [C, C], BF16, isOutput=False)
    wce1T = nc.declare_dram_parameter("wce1T", [C, HID], F32, isOutput=False)
    wce2T = nc.declare_dram_parameter("wce2T", [HID, 1], F32, isOutput=False)
    out = nc.declare_dram_parameter("out", [C, QN], F32, isOutput=True)

    # [ (t p) n ] -> [p t n] views for 128-partition SBUF tiles
    xb_r = xb.rearrange("(t p) n -> p t n", p=P)
    wq_r = wqT.rearrange("(t p) o -> p t o", p=P)
    wk_r = wkT.rearrange("(t p) o -> p t o", p=P)
    wv_r = wvT.rearrange("(t p) o -> p t o", p=P)
    wp_r = wpT.rearrange("(t p) o -> p t o", p=P)
    wce1_r = wce1T.rearrange("(t p) h -> p t h", p=P)
    out_r = out.rearrange("(t p) n -> t p n", p=P)

    et_dt = BF16 if bf16_av else F8

    with tile.TileContext(nc) as tc:
        with (
            tc.tile_pool(name="cst", bufs=1) as cst,
            tc.tile_pool(name="ework", bufs=1) as ework,
            tc.tile_pool(name="work", bufs=2) as work,
            tc.tile_pool(name="ps", bufs=1, space="PSUM") as psum,
        ):
            _loop = ExitStack()
            if reps > 1:
                _loop.enter_context(tc.For_i(0, reps))
            # ---- weight loads ----
            wq_sb = cst.tile([P, CT, C], BF16)
            nc.sync.dma_start(wq_sb[:], wq_r[:])
            wk_sb = cst.tile([P, CT, C], BF16)
            nc.sync.dma_start(wk_sb[:], wk_r[:])
            wv_sb = cst.tile([P, CT, C], BF16)
            nc.sync.dma_start(wv_sb[:], wv_r[:])
            wp_sb = cst.tile([P, CT, C], BF16)
            nc.sync.dma_start(wp_sb[:], wp_r[:])
            wce1_sb = cst.tile([P, CT, HID], F32)
            nc.sync.dma_start(wce1_sb[:], wce1_r[:])
            wce2_sb = cst.tile([HID, 1], F32)
            nc.sync.dma_start(wce2_sb[:], wce2T[:])

            # fp8 ones pair for the denominator matmuls (pair stride must be
            # a multiple of 16B, hence the padded free dim)
            ones8 = cst.tile([P, 2, 16], F8)
            nc.vector.memset(ones8[:], 1.0)
            ones_bf = cst.tile([P, 1], BF16)
            nc.vector.memset(ones_bf[:], 1.0)
            onesrow = cst.tile([1, P], BF16)
            nc.vector.memset(onesrow[:], 1.0)
            ebias = cst.tile([P, 1], F32)
            nc.vector.memset(ebias[:], EXP_BIAS)

            # ---- x stream-in (bf16) + per-chunk gate stats ----
            NXC = 8
            XCH = N // NXC
            xb_sb = cst.tile([P, CT, N], BF16)
            xsum8 = cst.tile([P, CT, NXC], F32)
            xmax8 = cst.tile([P, CT, NXC], F32)
            feat = cst.tile([P, CT], F32)
            for j in range(NXC):
                for t in range(CT):
                    xeng = nc.gpsimd if t == 1 else nc.sync
                    xeng.dma_start(
                        xb_sb[:, t, ts(j, XCH)], xb_r[:, t, ts(j, XCH)]
                    )
                    nc.vector.reduce_sum(
                        xsum8[:, t, j:j + 1], xb_sb[:, t, ts(j, XCH)],
                        axis=mybir.AxisListType.X,
                    )
                    nc.vector.reduce_max(
                        xmax8[:, t, j:j + 1], xb_sb[:, t, ts(j, XCH)],
                        axis=mybir.AxisListType.X,
                    )
            for t in range(CT):
                xm = work.tile([P, 1], F32, tag="xm")
                nc.vector.reduce_max(
                    xm[:], xmax8[:, t, :], axis=mybir.AxisListType.X
                )
                xs = work.tile([P, 1], F32, tag="xs")
                nc.vector.reduce_sum(
                    xs[:], xsum8[:, t, :], axis=mybir.AxisListType.X
                )
                nc.vector.scalar_tensor_tensor(
                    feat[:, t:t + 1], xs[:], 1.0 / N, xm[:],
                    op0=ALU.mult, op1=ALU.add,
                )

            # ---- gate MLP (tiny) ----
            ph = psum.tile([HID, 1], F32, tag="pr", bufs=1, name="ph")
            for t in range(CT):
                nc.tensor.matmul(
                    ph[:], wce1_sb[:, t, :], feat[:, t:t + 1],
                    start=(t == 0), stop=(t == CT - 1),
                )
            # silu/sigmoid via exp (stays on the ACT Exp table):
            # sigmoid(z) = 1/(1+exp(-z))
            e1 = cst.tile([HID, 1], F32)
            nc.scalar.activation(e1[:], ph[:], AF.Exp, scale=-1.0)
            d1 = cst.tile([HID, 1], F32)
            nc.vector.tensor_scalar_add(d1[:], e1[:], 1.0)
            r1 = cst.tile([HID, 1], F32)
            nc.vector.reciprocal(r1[:], d1[:])
            hid_sb = cst.tile([HID, 1], F32)
            nc.vector.tensor_tensor(hid_sb[:], ph[:], r1[:], op=ALU.mult)
            pc = psum.tile([1, 1], F32, tag="pr", bufs=1, name="pc")
            nc.tensor.matmul(pc[:], wce2_sb[:], hid_sb[:])
            e2 = cst.tile([1, 1], F32)
            nc.scalar.activation(e2[:], pc[:], AF.Exp, scale=-1.0)
            d2 = cst.tile([1, 1], F32)
            nc.vector.tensor_scalar_add(d2[:], e2[:], 1.0)
            cmplx = cst.tile([1, 1], F32)
            nc.vector.reciprocal(cmplx[:], d2[:])

            # ---- Q then K (bf16 matmuls -> fp8 casts on DVE) ----
            q8 = cst.tile([P, CT, QN], F8)
            k8 = cst.tile([P, CT, N], F8)
            v8 = cst.tile([P, MT, C], F8 if not bf16_av else BF16)

            def q_chunk(j):
                for t in range(CT):
                    pq = psum.tile([P, QCH], F32, tag="po", bufs=2, name="pq")
                    for kc in range(CT):
                        nc.tensor.matmul(
                            pq[:], wq_sb[:, kc, ts(t, P)],
                            xb_sb[:, kc, ts(j, QCH)],
                            start=(kc == 0), stop=(kc == CT - 1),
                        )
                    nc.vector.tensor_copy(q8[:, t, ts(j, QCH)], pq[:])

            q_chunk(0)
            for j in range(N // QCH):
                for t in range(CT):
                    pk = psum.tile([P, QCH], F32, tag="po", bufs=2, name="pk")
                    for kc in range(CT):
                        nc.tensor.matmul(
                            pk[:], wk_sb[:, kc, ts(t, P)],
                            xb_sb[:, kc, ts(j, QCH)],
                            start=(kc == 0), stop=(kc == CT - 1),
                        )
                    nc.vector.tensor_copy(k8[:, t, ts(j, QCH)], pk[:])
            for j in range(1, NQC):
                q_chunk(j)

            # ---- attention: 3-stage pipeline over query chunks ----
            # stage A (qc):   S matmuls (fp8 DoubleRow) + exp on ACT
            # stage B (qc-1): AV + denominator matmuls; normalize at end
            # stage C (qc-2): projection + residual + DMA out
            # chunk 0's stage-B slots run the V matmuls instead.
            o_sb = cst.tile([P, CT, QN], BF16)
            et_tiles = {}
            po_tiles = {}
            recipB_tiles = {}
            rr2_tiles = {}
            tree_tiles = {}

            def emit_v_slot(mp):
                # two token-tiles' V matmuls in chunk 0's AV slot mp
                pv2 = psum.tile([P, 2, C], F32, tag="po", bufs=2, name="pv")
                for hh in range(2):
                    nt = 2 * mp + hh
                    for kc in range(CT):
                        nc.tensor.matmul(
                            pv2[:, hh, :], xb_sb[:, kc, ts(nt, P)],
                            wv_sb[:, kc, :],
                            start=(kc == 0), stop=(kc == CT - 1),
                        )
                veng = nc.scalar if v_act else nc.vector
                if v_act:
                    veng.activation(v8[:, 2 * mp:2 * mp + 2, :], pv2[:],
                                    AF.Copy)
                else:
                    veng.tensor_copy(v8[:, 2 * mp:2 * mp + 2, :], pv2[:])

            for qc in range(NQC + 2):
                prev = qc - 1
                prev2 = qc - 2
                for mp in range(MT // 2):
                    # ---- stage A: S + exp for chunk qc ----
                    if qc < NQC and not no_attn:
                        pstg = psum.tile([P, 2, QCH], F32, tag="ps2",
                                         bufs=pstg_bufs)
                        for h in range(2):
                            mt = 2 * mp + h
                            if bf16_av:
                                nc.tensor.matmul(
                                    pstg[:, h, :], k8[:, :, ts(mt, P)],
                                    q8[:, :, ts(qc, QCH)],
                                    start=True, stop=True, perf_mode=DR,
                                )
                            else:
                                nc.tensor.matmul(
                                    pstg[:, h, :], k8[:, :, ts(mt, P)],
                                    q8[:, :, ts(qc, QCH)],
                                    start=True, stop=True, perf_mode=DR,
                                )
                        if mp == 0:
                            et_tiles[qc] = ework.tile(
                                [P, MT // 4, 2, 2, QCH], et_dt,
                                tag="exp", bufs=2, name="et",
                            )
                            if bf16_av:
                                tree_tiles[qc] = work.tile(
                                    [P, MT // 2, QCH], BF16, tag="tree",
                                    bufs=2, name="tree16",
                                )
                        et = et_tiles[qc]
                        nc.scalar.activation(
                            et[:, mp // 2, mp % 2, :, :], pstg[:], AF.Exp,
                            bias=ebias[:], scale=float(SCALE),
                        )
                        if bf16_av:
                            # pairwise level-0 add for the DVE tree
                            nc.vector.tensor_tensor(
                                tree_tiles[qc][:, mp, :],
                                et[:, mp // 2, mp % 2, 0, :],
                                et[:, mp // 2, mp % 2, 1, :], op=ALU.add,
                            )
                    # ---- stage B: AV (+denominator) for chunk prev ----
                    if qc == 0:
                        emit_v_slot(mp)
                    elif prev < NQC and not no_attn:
                        etp = et_tiles[prev]
                        if mp == 0:
                            po_tiles[prev] = [
                                psum.tile([P, QCH], F32, tag="po", bufs=2,
                                          name=f"po{ct}")
                                for ct in range(CT)
                            ]
                        for ct in range(CT):
                            if bf16_av:
                                for h in range(2):
                                    mt = 2 * mp + h
                                    nc.tensor.matmul(
                                        po_tiles[prev][ct][:],
                                        v8[:, mt, ts(ct, P)],
                                        etp[:, mp // 2, mp % 2, h, :],
                                        start=(mt == 0), stop=(mt == MT - 1),
                                        skip_group_check=True,
                                    )
                            else:
                                nc.tensor.matmul(
                                    po_tiles[prev][ct][:],
                                    v8[:, 2 * mp:2 * mp + 2, ts(ct, P)],
                                    etp[:, mp // 2, mp % 2, :, :],
                                    start=(mp == 0), stop=(mp == MT // 2 - 1),
                                    perf_mode=DR, skip_group_check=True,
                                )
                        if not bf16_av:
                            if mp == 0:
                                po_tiles[prev].append(
                                    psum.tile([1, QCH], F32, tag="pr",
                                              bufs=1, name="pr")
                                )
                            nc.tensor.matmul(
                                po_tiles[prev][2][:], ones8[:, :, 0:1],
                                etp[:, mp // 2, mp % 2, :, :],
                                start=(mp == 0), stop=(mp == MT // 2 - 1),
                                perf_mode=DR, skip_group_check=True,
                            )
                    # ---- mid-iteration stage-B/C glue ----
                    if mp == 2 and 0 <= prev < NQC and bf16_av and not no_attn:
                        # upper tree levels for prev: 16 -> 8 -> 4 -> 2 -> 1
                        tr = tree_tiles[prev]
                        w_half = MT // 4
                        while w_half >= 1:
                            nc.vector.tensor_tensor(
                                tr[:, :w_half, :], tr[:, :w_half, :],
                                tr[:, w_half:2 * w_half, :], op=ALU.add,
                            )
                            w_half //= 2
                        prt = psum.tile([1, QCH], F32, tag="pr", bufs=1,
                                        name="pr")
                        po_tiles[prev].append(prt)
                        nc.tensor.matmul(prt[:], ones_bf[:], tr[:, 0, :])
                    if mp == 10 and 0 <= prev2 < NQC and not no_attn:
                        # stage C: projection + residual + out for prev2
                        for ct in range(CT):
                            pp = psum.tile([P, QCH], F32, tag="pp", bufs=1)
                            for kc in range(CT):
                                nc.tensor.matmul(
                                    pp[:], wp_sb[:, kc, ts(ct, P)],
                                    o_sb[:, kc, ts(prev2, QCH)],
                                    start=(kc == 0), stop=(kc == CT - 1),
                                )
                            outt = work.tile([P, QCH], F32, tag="outt",
                                             bufs=3)
                            nc.vector.tensor_tensor(
                                outt[:], pp[:],
                                xb_sb[:, ct, ts(prev2, QCH)], op=ALU.add,
                            )
                            nc.sync.dma_start(
                                out_r[ct, :, ts(prev2, QCH)], outt[:]
                            )
                # ---- end of iteration: denominator -> normalize prev ----
                if 0 <= prev < NQC and not no_attn:
                    prt = po_tiles[prev][2]
                    rr = work.tile([1, QCH], F32, tag="rr")
                    nc.vector.reciprocal(rr[:], prt[:])
                    rr2 = work.tile([1, QCH], BF16, tag="rr2", bufs=2)
                    nc.vector.tensor_scalar_mul(rr2[:], rr[:], cmplx[:1, :1])
                    rr2_tiles[prev] = rr2
                    pb = psum.tile([P, QCH], F32, tag="pr", bufs=1, name="pb")
                    nc.tensor.matmul(pb[:], onesrow[:], rr2[:])
                    recipB = work.tile([P, QCH], F32, tag="recipB", bufs=2)
                    nc.scalar.activation(recipB[:], pb[:], AF.Copy)
                    recipB_tiles[prev] = recipB
                    for ct in range(CT):
                        nc.vector.tensor_tensor(
                            o_sb[:, ct, ts(prev, QCH)],
                            po_tiles[prev][ct][:], recipB[:], op=ALU.mult,
                        )
            _loop.close()

    nc.finalize()
    return nc


_NC_CACHE = {}


def _get_nc():
    if "nc" not in _NC_CACHE:
        _NC_CACHE["nc"] = _build()
    return _NC_CACHE["nc"]


def _in_maps(x, w_ce1, w_ce2, wq, wk, wv, wproj):
    x = np.asarray(x, dtype=np.float32)
    wqT = np.ascontiguousarray(np.asarray(wq, np.float32).T).astype(bf16)
    wkT = np.ascontiguousarray(np.asarray(wk, np.float32).T).astype(bf16)
    wvT = np.ascontiguousarray(np.asarray(wv, np.float32).T).astype(bf16)
    wpT = np.ascontiguousarray(np.asarray(wproj, np.float32).T).astype(bf16)
    wce1T = np.ascontiguousarray(np.asarray(w_ce1, np.float32).T)
    wce2T = np.ascontiguousarray(np.asarray(w_ce2, np.float32).T)
    maps = []
    for c in range(NCORES):
        b, h = divmod(c, NCORES // B)
        xf = x[b].reshape(C, N)
        # keys ordered [my half | other half]; attention is permutation-
        # invariant over keys so any consistent order works
        xc = np.concatenate(
            [xf[:, h * QN:(h + 1) * QN], xf[:, (1 - h) * QN:(2 - h) * QN]],
            axis=1,
        )
        maps.append({
            "xb": np.ascontiguousarray(xc).astype(bf16),
            "wqT": wqT, "wkT": wkT, "wvT": wvT, "wpT": wpT,
            "wce1T": wce1T, "wce2T": wce2T,
        })
    return maps


def kernel(x, w_ce1, w_ce2, wq, wk, wv, wproj):
    x = np.asarray(x, dtype=np.float32)
    assert x.shape == (B, C, H, W)
    in_maps = _in_maps(x, w_ce1, w_ce2, wq, wk, wv, wproj)
    res = run_bass_kernel_spmd(_get_nc(), in_maps, list(range(NCORES)))
    out = np.empty((B, C, N), dtype=np.float32)
    for c in range(NCORES):
        b, h = divmod(c, NCORES // B)
        out[b][:, h * QN:(h + 1) * QN] = res.results[c]["out"]
    return out.reshape(B, C, H, W)
